# revision 10
# baseline (speedup 1.0000x reference)
"""Trainium2 Bass kernel for nn_Attention_14113262534866.

Self-attention over 64x64 "pixels" (n=4096), batch=2, heads=4, dim_head=32.
Sharding: one (batch, head) pair per NeuronCore (8 cores). Each core:
  - projects its head's q/k/v from x[b]  (1x1 conv == channel matmul)
  - computes softmax(q^T k / sqrt(d)) @ v in a transposed layout
    (dots^T[j, i] with j on partitions) so no attention transpose is needed
  - applies its head's slice of the output projection -> a partial [256, 4096]
Host unshard: sum the 4 head-partials per batch and reshape (bias is applied
on-device as bias/4 per core so the sum reconstructs it).

All big matmuls use float32r (FP22 multiplies, fp32 accumulate): 1 cycle/row
on TensorE vs 4 for full fp32, ~6e-5 relative rounding. Inputs are
pre-rounded to FP22 on the host so walrus' "rounded producer" rule holds.
"""

import numpy as np

import concourse.mybir as mybir
import concourse.tile as tile
from concourse import bacc
from concourse.bass_utils import run_bass_kernel_spmd

F32 = mybir.dt.float32
F32R = mybir.dt.float32r
EXP = mybir.ActivationFunctionType.Exp

HEADS = 4
DIM_HEAD = 32
SCALE = DIM_HEAD ** -0.5
DIM = 256
N = 4096                 # 64*64 pixels
NB = 8                   # number of i-blocks
IB = 512                 # i-block width (one psum bank)
JT = 32                  # j chunks of 128
P = 128


def build_program():
    nc = bacc.Bacc(None, target_bir_lowering=False, debug=False)

    x_d = nc.declare_dram_parameter("x", [2, P, N], F32R, isOutput=False)
    wq_d = nc.declare_dram_parameter("wq", [P, 2, 64], F32R, isOutput=False)
    wk_d = nc.declare_dram_parameter("wk", [P, 2, 64], F32R, isOutput=False)
    wv_d = nc.declare_dram_parameter("wv", [P, 2, 32], F32R, isOutput=False)
    wo_d = nc.declare_dram_parameter("wo", [P, 256], F32R, isOutput=False)
    bias_d = nc.declare_dram_parameter("bias4", [P, 2], F32, isOutput=False)
    out_d = nc.declare_dram_parameter("out", [DIM, N], F32, isOutput=True)

    with tile.TileContext(nc) as tc:
        with (
            tc.tile_pool(name="const", bufs=1) as const,
            tc.tile_pool(name="qkv", bufs=1) as qkv,
            tc.tile_pool(name="attn", bufs=20) as attnp,
            tc.tile_pool(name="small", bufs=3) as small,
            tc.tile_pool(name="qk_ps", bufs=2, space="PSUM") as qk_ps,
            tc.tile_pool(name="av_ps", bufs=1, space="PSUM") as av_ps,
            tc.tile_pool(name="pj_ps", bufs=3, space="PSUM") as pj_ps,
        ):
            # ---- constants / inputs to SBUF ----
            x_sb = [const.tile([P, N], F32R, tag=f"x{c}", name=f"x_sb{c}")
                    for c in range(2)]
            for c in range(2):
                nc.sync.dma_start(x_sb[c][:], x_d[c])
            wq_sb = const.tile([P, 2, 64], F32R, tag="wq")
            wk_sb = const.tile([P, 2, 64], F32R, tag="wk")
            wv_sb = const.tile([P, 2, 32], F32R, tag="wv")
            wo_sb = const.tile([P, 256], F32R, tag="wo")
            bias_sb = const.tile([P, 2], F32, tag="bias")
            nc.sync.dma_start(wq_sb[:], wq_d[:])
            nc.sync.dma_start(wk_sb[:], wk_d[:])
            nc.sync.dma_start(wv_sb[:], wv_d[:])
            nc.sync.dma_start(wo_sb[:], wo_d[:])
            nc.sync.dma_start(bias_sb[:], bias_d[:])
            ones_f32 = const.tile([P, 1], F32, tag="ones_f32")
            ones_col = const.tile([P, 1], F32R, tag="ones_col")
            ones_row = const.tile([P, P], F32, tag="ones_row")
            nc.vector.memset(ones_f32[:], 1.0)
            nc.vector.memset(ones_row[:], 1.0)
            nc.vector.tensor_copy(ones_col[:], ones_f32[:])

            # ---- qkv projection ----
            # q_rep/k_rep: [64, N] with the head's [32, N] duplicated on
            # partition groups 0-31 / 32-63 (for 2x row-tiled QK matmuls).
            q_rep = qkv.tile([64, N], F32R, tag="q_rep")
            k_rep = qkv.tile([64, N], F32R, tag="k_rep")
            for dst, w_sb in ((q_rep, wq_sb), (k_rep, wk_sb)):
                for t in range(NB):
                    ps = qk_ps.tile([P, 2 * IB], F32, tag="qk", name="qk_ps_t")
                    for c in range(2):
                        nc.tensor.matmul(
                            ps[0:64, 0:IB],
                            lhsT=w_sb[:, c, :],
                            rhs=x_sb[c][:, t * IB:(t + 1) * IB],
                            start=(c == 0), stop=(c == 1),
                        )
                    nc.vector.tensor_copy(dst[:, t * IB:(t + 1) * IB],
                                          ps[0:64, 0:IB])

            # vT: [128, 32, 33]; vT[p, t, d] = v[d, 128t+p]; col 32 = 1.0
            # (the ones column makes the AV matmul also produce softmax sums)
            vT = qkv.tile([P, JT, 33], F32R, tag="vT")
            ones32_f32 = const.tile([P, JT], F32, tag="ones32")
            nc.vector.memset(ones32_f32[:], 1.0)
            nc.vector.tensor_copy(vT[:, :, 32], ones32_f32[:])
            for gp in range(8):
                ps = pj_ps.tile([P, IB], F32, tag="pj", name="pj_ps_t")
                for lane in range(4):
                    pt = 4 * gp + lane
                    for c in range(2):
                        nc.tensor.matmul(
                            ps[:, 32 * lane:32 * lane + 32],
                            lhsT=x_sb[c][:, pt * P:(pt + 1) * P],
                            rhs=wv_sb[:, c, :],
                            start=(c == 0), stop=(c == 1),
                        )
                nc.vector.tensor_copy(
                    vT[:, 4 * gp:4 * gp + 4, 0:32],
                    ps[:, 0:P].rearrange("p (l d) -> p l d", l=4),
                )

            # ---- main attention loop, software-pipelined over i-blocks ----
            attn_tiles = [[None] * 16 for _ in range(NB)]

            def qk_phase(ib):
                for g in range(16):  # pairs of j-chunks
                    ps = qk_ps.tile([P, 2 * IB], F32, tag="qk", name="qk_ps_t")
                    for half in range(2):  # 2x row-tiled (K=32)
                        jc = 2 * g + half
                        nc.tensor.matmul(
                            ps[:, half * IB:(half + 1) * IB],
                            lhsT=k_rep[32 * half:32 * half + 32,
                                       jc * P:(jc + 1) * P],
                            rhs=q_rep[32 * half:32 * half + 32,
                                      ib * IB:(ib + 1) * IB],
                            tile_position=(32 * half, 0),
                            start=True, stop=True,
                        )
                    at = attnp.tile([P, 2 * IB], F32R, tag="attn",
                                    name="attn_t")
                    nc.scalar.activation(at[:], ps[:], EXP)
                    attn_tiles[ib][g] = at

            def av_phase(ib):
                # single accumulation chain: rows 0-31 = attn @ v,
                # row 32 = softmax denominators (ones column of vT)
                av = av_ps.tile([P, IB], F32, tag="av", name="av_t")
                for g in range(16):
                    at = attn_tiles[ib][g]
                    for half in range(2):
                        jc = 2 * g + half
                        nc.tensor.matmul(
                            av[0:33, :],
                            lhsT=vT[:, jc, :],
                            rhs=at[:, half * IB:(half + 1) * IB],
                            tile_position=(0, 0),
                            start=(jc == 0), stop=(jc == 31),
                        )
                    attn_tiles[ib][g] = None

                sb = small.tile([33, IB], F32R, tag="hout", name="hout_t")
                nc.vector.tensor_copy(sb[:], av[0:33, :])

                # select the sums row down to partition 0 (K=1 matmul),
                # then 1/s there (reciprocal_approx_fast needs partition 0)
                sps = pj_ps.tile([P, IB], F32, tag="pj", name="s_ps_t")
                nc.tensor.matmul(sps[0:1, :], lhsT=ones_col[32:33, :],
                                 rhs=sb[32:33, :],
                                 tile_position=(32, 0), start=True, stop=True)
                recip = small.tile([1, IB], F32, tag="recip", name="recip_t")
                nc.vector.reciprocal_approx_fast(recip[:], sps[0:1, :])

                # broadcast 1/s to 128 partitions (K=1 fp32 matmul)
                bcp = pj_ps.tile([P, IB], F32, tag="pj", name="bc_ps_t")
                nc.tensor.matmul(bcp[:], lhsT=ones_row[0:1, 0:P],
                                 rhs=recip[0:1, :],
                                 tile_position=(0, 0), start=True, stop=True)
                bc = small.tile([P, IB], F32, tag="bc", name="bc_t")
                nc.vector.tensor_copy(bc[:], bcp[:])

                # output projection, then 1/s + bias
                for ot in range(2):
                    pj = pj_ps.tile([P, IB], F32, tag="pj", name="pj_t")
                    nc.tensor.matmul(pj[:],
                                     lhsT=wo_sb[0:32, ot * P:(ot + 1) * P],
                                     rhs=sb[0:32, :],
                                     tile_position=(0, 0),
                                     start=True, stop=True)
                    osb = small.tile([P, IB], F32, tag=f"osb{ot}",
                                     name="osb_t")
                    nc.vector.tensor_mul(osb[:], pj[:], bc[:])
                    nc.vector.tensor_scalar_add(osb[:], osb[:],
                                                bias_sb[:, ot:ot + 1])
                    nc.sync.dma_start(
                        out_d[ot * P:(ot + 1) * P, ib * IB:(ib + 1) * IB],
                        osb[:],
                    )

            # emit QK(ib) before AV(ib-1) so the PE stream stays ahead of
            # ScalarE (exp) and the engines pipeline across i-blocks
            for ib in range(NB + 1):
                if ib < NB:
                    qk_phase(ib)
                if ib >= 1:
                    av_phase(ib - 1)

    nc.compile()
    return nc


def to_fp22(a):
    """Round fp32 to FP22 (13-bit mantissa) — what the PE reads for f32r."""
    u = np.ascontiguousarray(a, np.float32).view(np.uint32)
    u = (u + 0x1FF + ((u >> 10) & 1)) & np.uint32(0xFFFFFC00)
    return u.view(np.float32)


def make_core_inputs(x, w_qkv, w_out, b_out, core):
    b, h = core // HEADS, core % HEADS
    xb = np.ascontiguousarray(x[b].reshape(DIM, N)).astype(np.float32)
    w_q = w_qkv[h * 32:(h + 1) * 32, :] * SCALE
    w_k = w_qkv[128 + h * 32:128 + (h + 1) * 32, :]
    w_v = w_qkv[256 + h * 32:256 + (h + 1) * 32, :]
    wqT = np.ascontiguousarray(w_q.T)          # [256, 32]
    wkT = np.ascontiguousarray(w_k.T)
    wvT = np.ascontiguousarray(w_v.T)
    # layouts match SBUF tiles: [partition, c_chunk, m]
    wq_in = np.stack([np.tile(wqT[c * P:(c + 1) * P], (1, 2))
                      for c in range(2)], axis=1)
    wk_in = np.stack([np.tile(wkT[c * P:(c + 1) * P], (1, 2))
                      for c in range(2)], axis=1)
    wv_in = np.stack([wvT[c * P:(c + 1) * P] for c in range(2)], axis=1)
    woT = np.ascontiguousarray(w_out[:, h * 32:(h + 1) * 32].T)  # [32, 256]
    wo_in = np.zeros((P, 256), np.float32)
    wo_in[0:32] = woT
    wo_in[64:96] = woT
    bias_in = np.ascontiguousarray(b_out.reshape(2, P).T / HEADS)
    return {
        "x": to_fp22(xb.reshape(2, P, N)),
        "wq": to_fp22(wq_in),
        "wk": to_fp22(wk_in),
        "wv": to_fp22(wv_in),
        "wo": to_fp22(wo_in),
        "bias4": bias_in.astype(np.float32),
    }


_NC_CACHE = []


def get_nc():
    if not _NC_CACHE:
        _NC_CACHE.append(build_program())
    return _NC_CACHE[0]


def run(inputs, trace=False, tmpdir=None):
    nc = get_nc()
    in_maps = [
        make_core_inputs(inputs["x"], inputs["w_qkv"], inputs["w_out"],
                         inputs["b_out"], core)
        for core in range(8)
    ]
    kw = {}
    if trace:
        kw = dict(trace=True, tmpdir=tmpdir)
    res = run_bass_kernel_spmd(nc, in_maps, list(range(8)), **kw)
    b = inputs["x"].shape[0]
    hh, ww = inputs["x"].shape[2], inputs["x"].shape[3]
    out = np.zeros((b, DIM, hh, ww), np.float32)
    for bb in range(b):
        acc = np.zeros((DIM, N), np.float32)
        for h in range(HEADS):
            acc += res.results[bb * HEADS + h]["out"]
        out[bb] = acc.reshape(DIM, hh, ww)
    return out, res


def kernel(**inputs):
    out, _ = run(inputs)
    return out
